# revision 37
# baseline (speedup 1.0000x reference)
"""CAMSA multi-mask attention kernel for one TRN2 chip (8 NeuronCores).

Problem: B=4, S=2048, D=1024, M=4 stride masks.
  Q = x@Wq ; K = x@Wk ; V = x@Wv      (biases zero-fill)
  scores = Q K^T / sqrt(D)
  weights_m = softmax(where(mask_m==0, -1e9, scores))
  out = (mean_m weights_m) @ V @ Wo

Algebra (v2): Q/K/V are never materialized.
  G = Wq Wk^T ; H = Wv Wo            (weight folding, [D,D] each)
  scores = (x G) x^T                 -> T = xq@G, scores = T x^T
  out    = (Wsum x) H                -> U = Wsum@x, out = U H
This removes the K and V projections entirely: per-core matmul work
drops from 273us to 218us (G,H,T cost 82us vs Q,K,V's 137us + the
attention matmuls run on x instead of K/V).

P = exp(scores/sqrt(D))/M (the 1/M folds into the exp bias); per-mask
  den_m[q] = sum_k mask_m[q,k] P[q,k];  inv_m = 1/den_m
  Wsum = sum_m inv_m * (mask_m*P);  out = Wsum @ x @ H

Sharding: core c = (batch b=c//2, query-half h=c%2): 1024 query rows,
full 2048 keys; no collectives. Masks are host-packed to uint8 and
weights/x host-cast to bf16 (DMA per core ~26MB vs 80MB in v1).

Engine split per q-tile (the v1 all-DVE softmax path was the limiter;
DVE STT has no 2x perf mode):
  den products: 2 on DVE STT + 2 on GpSimd STT (both with accum row-sum)
  combine: 2 ACT scale-copies + 1 DVE STT + 1 GpSimd STT + 1 DVE TT-add
  exp: ACT (bias = -ln 4 folds the 1/M)
"""

import numpy as np

B, S, D, M = 4, 2048, 1024, 4
SQ = S // 2          # query rows per core
PART = 128
N_CORES = 8

_CACHE = {}


def build(nc_factory=None, use_deps=True):
    from concourse import bass, mybir, bacc, tile
    from concourse.tile import add_dep_helper

    fp32 = mybir.dt.float32
    bf16 = mybir.dt.bfloat16
    u8 = mybir.dt.uint8
    AF = mybir.ActivationFunctionType
    ALU = mybir.AluOpType

    P = PART
    DCH = D // P         # d-chunks (8)
    KCH = S // P         # key-row chunks (16)
    QTILES = SQ // P     # q-tiles per core (8)
    NB = 512

    if nc_factory is None:
        nc = bacc.Bacc("TRN2", target_bir_lowering=False, debug=False,
                       num_devices=N_CORES)
    else:
        nc = nc_factory()

    xT_d = nc.dram_tensor("xT", [D, S], bf16, kind="ExternalInput")
    xTq_d = nc.dram_tensor("xTq", [D, SQ], bf16, kind="ExternalInput")
    xk_d = nc.dram_tensor("xk", [S, D], bf16, kind="ExternalInput")
    mk_d = nc.dram_tensor("mk", [M, SQ, S], u8, kind="ExternalInput")
    wqt_d = nc.dram_tensor("wqt", [D, D], bf16, kind="ExternalInput")
    wkt_d = nc.dram_tensor("wkt", [D, D], bf16, kind="ExternalInput")
    wvt_d = nc.dram_tensor("wvt", [D, D], bf16, kind="ExternalInput")
    wo_d = nc.dram_tensor("wo", [D, D], bf16, kind="ExternalInput")
    out_d = nc.dram_tensor("out", [SQ, D], fp32, kind="ExternalOutput")

    ncopy = [0]

    with tile.TileContext(nc) as tc:
        with tc.tile_pool(name="persist", bufs=1) as pp, \
             tc.tile_pool(name="psum", bufs=8, space="PSUM") as psp:

            xTs = pp.tile([P, DCH * S], bf16)    # [p, c*S+k]  = xT[c*128+p, k]
            xks = pp.tile([P, KCH * D], bf16)    # [p, i*D+d]  = x[i*128+p, d]
            TT = pp.tile([P, DCH * SQ], bf16)    # [p, c*SQ+q] = T[q, c*128+p]
            Hs = pp.tile([P, DCH * D], bf16)     # [p, c*D+o]  = H[c*128+p, o]
            WT = pp.tile([P, KCH * SQ], bf16)    # [p, i*SQ+q] = Wsum[q, i*128+p]
            OT = pp.tile([P, DCH * NB], bf16)    # [p, c*NB+qc] = U[qb*NB+qc, c*128+p]

            def drain(dst, ps):
                # PSUM -> SBUF copies, alternating DVE / ACT
                ncopy[0] += 1
                if ncopy[0] % 2:
                    nc.vector.tensor_copy(dst, ps)
                else:
                    nc.scalar.activation(dst, ps, AF.Copy, scale=1.0)

            def wload(dst, src_d, eng=None):
                return (eng or nc.gpsimd).dma_start(
                    dst[:].rearrange("p (c d) -> p c d", c=DCH),
                    src_d.ap().rearrange("(c p) d -> p c d", p=P))

            # ---- phase B: G = Wq Wk^T ; TT = (xq G)^T ; H = Wv Wo --------
            with tc.tile_pool(name="stage", bufs=1) as sw:
                wa = sw.tile([P, DCH * D], bf16, name="wa", tag="wa")
                wb = sw.tile([P, DCH * D], bf16, name="wb", tag="wb")
                Gs = sw.tile([P, DCH * D], bf16, name="Gs")
                xTq = sw.tile([P, DCH * SQ], bf16, name="xTq")

                def wload_half(dst, src_d, half):
                    lo, hi = half * (D // 2), (half + 1) * (D // 2)
                    return nc.gpsimd.dma_start(
                        dst[:].rearrange("p (c d) -> p c d", c=DCH)[:, :, lo:hi],
                        src_d.ap()[:, lo:hi].rearrange("(c p) d -> p c d", p=P))

                # halves so G's first chains start after ~6us of DMA
                d_wq = wload_half(wa, wqt_d, 0)
                d_wk = wload_half(wb, wkt_d, 0)
                d_wq1 = wload_half(wa, wqt_d, 1)
                d_wk1 = wload_half(wb, wkt_d, 1)
                d_xq = nc.gpsimd.dma_start(
                    xTq[:].rearrange("p (c r) -> p c r", c=DCH),
                    xTq_d.ap().rearrange("(c p) r -> p c r", p=P))
                d_xt = nc.gpsimd.dma_start(
                    xTs[:].rearrange("p (c r) -> p c r", c=DCH),
                    xT_d.ap().rearrange("(c p) r -> p c r", p=P))
                # second use of the stage slots: WvT / Wo after G is done
                wa2 = sw.tile([P, DCH * D], bf16, name="wa2", tag="wa")
                wb2 = sw.tile([P, DCH * D], bf16, name="wb2", tag="wb")
                d_wv = wload(wa2, wvt_d)
                d_wo = wload(wb2, wo_d)
                d_xk = nc.gpsimd.dma_start(
                    xks[:].rearrange("p (i d) -> p i d", i=KCH),
                    xk_d.ap().rearrange("(i p) d -> p i d", p=P))
                if use_deps:
                    for a, b in zip(
                            [d_wk, d_wq1, d_wk1, d_xq, d_xt, d_wv, d_wo, d_xk],
                            [d_wq, d_wk, d_wq1, d_wk1, d_xq, d_xt, d_wv, d_wo]):
                        add_dep_helper(a.ins, b.ins, sync=False, reason="dma order")

                # G[i,j] = sum_d Wq[i,d] Wk[j,d]: lhsT=WqT chunk, rhs=WkT
                # (jb-outer + ic split so the first chains only need the
                # first column-halves of wqt/wkt)
                for jb in range(D // NB):
                    for ic in range(DCH):
                        ps = psp.tile([P, NB], fp32, tag="ps", name="ps")
                        for c in range(DCH):
                            nc.tensor.matmul(
                                ps[:],
                                wa[:, c * D + ic * P: c * D + (ic + 1) * P],
                                wb[:, c * D + jb * NB: c * D + (jb + 1) * NB],
                                start=(c == 0), stop=(c == DCH - 1))
                        drain(Gs[:, ic * D + jb * NB: ic * D + (jb + 1) * NB], ps[:])
                # TT[j,q] = sum_i G[i,j] xq[q,i]: lhsT=G chunk, rhs=xTq
                for jc in range(DCH):
                    for qb in range(SQ // NB):
                        ps = psp.tile([P, NB], fp32, tag="ps", name="ps")
                        for ic in range(DCH):
                            nc.tensor.matmul(
                                ps[:],
                                Gs[:, ic * D + jc * P: ic * D + (jc + 1) * P],
                                xTq[:, ic * SQ + qb * NB: ic * SQ + (qb + 1) * NB],
                                start=(ic == 0), stop=(ic == DCH - 1))
                        drain(TT[:, jc * SQ + qb * NB: jc * SQ + (qb + 1) * NB], ps[:])
                # H[i,o] = sum_d Wv[i,d] Wo[d,o]: lhsT=WvT chunk, rhs=Wo
                for ic in range(DCH):
                    for ob in range(D // NB):
                        ps = psp.tile([P, NB], fp32, tag="ps", name="ps")
                        for c in range(DCH):
                            nc.tensor.matmul(
                                ps[:],
                                wa2[:, c * D + ic * P: c * D + (ic + 1) * P],
                                wb2[:, c * D + ob * NB: c * D + (ob + 1) * NB],
                                start=(c == 0), stop=(c == DCH - 1))
                        drain(Hs[:, ic * D + ob * NB: ic * D + (ob + 1) * NB], ps[:])

            # ---- phases C/E/F/G ----------------------------------------
            wk_ctx = tc.tile_pool(name="work", bufs=2)
            wkp = wk_ctx.__enter__()

            def mt_load(t):
                # gpsimd queue: gp does no elementwise work in phase C, so
                # its queue is free for the mask DMA triggers
                mt = wkp.tile([P, M * S], u8, tag="mt", name=f"mt{t}", bufs=2)
                nc.gpsimd.dma_start(
                    mt[:].rearrange("p (m k) -> p m k", m=M),
                    mk_d.ap()[:, t * P:(t + 1) * P, :].transpose([1, 0, 2]))
                return mt

            inv_scale = 1.0 / float(np.sqrt(np.float32(D)))
            mts = {0: mt_load(0), 1: mt_load(1)}
            pts = {}

            def sc_exp(t):
                """scores tile -> exp -> Pt (emitted 2 tiles ahead so the
                ACT queue's exp isn't gated by the previous tile's scales)."""
                Pt = wkp.tile([P, S], bf16, tag="Pt", name="Pt", bufs=4)
                for kb in range(S // NB):
                    ps = psp.tile([P, NB], fp32, tag="ps", name="ps")
                    for c in range(DCH):
                        nc.tensor.matmul(
                            ps[:],
                            TT[:, c * SQ + t * P: c * SQ + (t + 1) * P],
                            xTs[:, c * S + kb * NB: c * S + (kb + 1) * NB],
                            start=(c == 0), stop=(c == DCH - 1))
                    nc.scalar.activation(
                        Pt[:, kb * NB:(kb + 1) * NB], ps[:],
                        AF.Exp, scale=inv_scale)
                pts[t] = Pt

            def c_prod(t):
                """tile t products: fused mask*P + row-sums, all on DVE.
                GpSimd does NO elementwise work in phase C: it shares SBUF
                ports with DVE, so concurrent gp ops halve DVE throughput.
                (The softmax-mean's 1/M is folded into wvt on the host.)"""
                if t + 2 < QTILES:
                    mts[t + 2] = mt_load(t + 2)
                mt = mts.pop(t)
                Pt = pts.pop(t)

                den = wkp.tile([P, M], fp32, tag="den", name="den")
                Tm = [wkp.tile([P, S], bf16, tag=f"Tm{m}", name=f"Tm{m}",
                               bufs=2)
                      for m in range(M)]
                for m in range(M):
                    nc.vector.scalar_tensor_tensor(
                        out=Tm[m][:],
                        in0=mt[:, m * S:(m + 1) * S],
                        scalar=1.0, in1=Pt[:],
                        op0=ALU.mult, op1=ALU.mult,
                        accum_out=den[:, m:m + 1])
                return den, Tm

            def c_norm(t, den, Tm):
                """recip + per-mask inv scaling on ACT (own SBUF ports)."""
                inv = wkp.tile([P, M], fp32, tag="inv", name="inv")
                nc.vector.reciprocal(inv[:], den[:])
                for m in range(M):
                    nc.scalar.activation(Tm[m][:], Tm[m][:], AF.Copy,
                                         scale=inv[:, m:m + 1])
                return Tm

            def c_tail(t, Tm):
                nc.vector.tensor_add(Tm[0][:], Tm[0][:], Tm[1][:])
                nc.vector.tensor_add(Tm[2][:], Tm[2][:], Tm[3][:])
                nc.vector.tensor_add(Tm[0][:], Tm[0][:], Tm[2][:])
                # transpose Wsum [128, S] -> WT column t via xbar DMA
                nc.sync.dma_start_transpose(
                    WT[:].rearrange("p (i q) -> p i q", i=KCH)
                    [:, :, t * P:(t + 1) * P],
                    Tm[0][:])

            def f_block(qb):
                # OT[j, qc] = sum_k x[k, j*128+jj] Wsum[qb*NB+qc, k]
                for j in range(DCH):
                    ps = psp.tile([P, NB], fp32, tag="ps", name="ps")
                    for i in range(KCH):
                        nc.tensor.matmul(
                            ps[:],
                            xks[:, i * D + j * P: i * D + (j + 1) * P],
                            WT[:, i * SQ + qb * NB: i * SQ + (qb + 1) * NB],
                            start=(i == 0), stop=(i == KCH - 1))
                    drain(OT[:, j * NB:(j + 1) * NB], ps[:])

            def g_tile(t):
                ot = wkp.tile([P, D], fp32, tag="ot", name="ot", bufs=1)
                for ob in range(D // NB):
                    ps = psp.tile([P, NB], fp32, tag="ps", name="ps")
                    for c in range(DCH):
                        nc.tensor.matmul(
                            ps[:],
                            OT[:, c * NB + (t % 4) * P: c * NB + (t % 4 + 1) * P],
                            Hs[:, c * D + ob * NB: c * D + (ob + 1) * NB],
                            start=(c == 0), stop=(c == DCH - 1))
                    drain(ot[:, ob * NB:(ob + 1) * NB], ps[:])
                nc.sync.dma_start(out_d.ap()[t * P:(t + 1) * P, :], ot[:])

            # software pipeline: products(t) | tail(t-1) | norm(t) | exp(t+2)
            # - per-queue order keeps every engine's next op data-ready
            sc_exp(0)
            sc_exp(1)
            prev = None
            for t in range(QTILES):
                den, Tm = c_prod(t)
                if prev is not None:
                    c_tail(t - 1, prev)
                prev = c_norm(t, den, Tm)
                if t + 2 < QTILES:
                    sc_exp(t + 2)
            c_tail(QTILES - 1, prev)
            for qb in range(SQ // NB):
                f_block(qb)
                for t in range(qb * (NB // P), (qb + 1) * (NB // P)):
                    g_tile(t)
            wk_ctx.__exit__(None, None, None)

    nc.compile()
    return nc


def _get_nc():
    if "nc" not in _CACHE:
        _CACHE["nc"] = build()
    return _CACHE["nc"]


def kernel(x, stride_masks, Wq, bq, Wk, bk, Wv, bv, Wo, bo):
    import ml_dtypes
    from concourse import bass_utils

    bf16 = ml_dtypes.bfloat16
    x = np.ascontiguousarray(np.asarray(x, dtype=np.float32))
    stride_masks = np.asarray(stride_masks, dtype=np.int32)
    Wq = np.asarray(Wq, dtype=np.float32)
    Wk = np.asarray(Wk, dtype=np.float32)
    Wv = np.asarray(Wv, dtype=np.float32)
    Wo = np.asarray(Wo, dtype=np.float32)
    bq = np.asarray(bq, dtype=np.float32)
    bk = np.asarray(bk, dtype=np.float32)
    bv = np.asarray(bv, dtype=np.float32)
    bo = np.asarray(bo, dtype=np.float32)

    nc = _get_nc()

    # Biases are spec'd zero-fill; the device kernel omits them. bv/bo fold
    # in exactly on the host (softmax rows sum to 1); bq/bk would need a
    # device path, so assert they are zero.
    assert not (np.any(bq) or np.any(bk)), "nonzero q/k bias unsupported"

    mk_u8 = stride_masks.astype(np.uint8)
    mk_half = [np.ascontiguousarray(mk_u8[:, h * SQ:(h + 1) * SQ, :])
               for h in range(2)]
    wqt = Wq.T.astype(bf16)
    wkt = Wk.T.astype(bf16)
    wvt = (Wv.T / np.float32(M)).astype(bf16)   # folds the mask-mean 1/M
    wo16 = Wo.astype(bf16)
    xT_bf = [x[b].T.astype(bf16) for b in range(B)]
    xk_bf = [x[b].astype(bf16) for b in range(B)]

    in_maps = []
    for c in range(N_CORES):
        b, h = c // 2, c % 2
        in_maps.append({
            "xT": xT_bf[b],
            "xTq": np.ascontiguousarray(xT_bf[b][:, h * SQ:(h + 1) * SQ]),
            "xk": xk_bf[b], "mk": mk_half[h],
            "wqt": wqt, "wkt": wkt, "wvt": wvt, "wo": wo16,
        })

    res = bass_utils.run_bass_kernel_spmd(nc, in_maps, core_ids=list(range(N_CORES)))
    _CACHE["last_results"] = res

    out = np.empty((B, S, D), dtype=np.float32)
    for c in range(N_CORES):
        b, h = c // 2, c % 2
        out[b, h * SQ:(h + 1) * SQ, :] = res.results[c]["out"]

    if np.any(bv):
        out += (bv @ Wo)[None, None, :]
    if np.any(bo):
        out += bo[None, None, :]
    return out
